# revision 1
# baseline (speedup 1.0000x reference)
"""EntityAttentionLayer on 8 Trainium2 NeuronCores.

Data-parallel over batch (16 batches/core). All matmuls bf16 with f32 PSUM
accumulation. Layouts are chosen so no PE transpose of activations is needed:
q/k are produced pre-transposed by the projection layout, v naturally, and
the attention output is transposed by the DMA xbar on otherwise-idle DMA
engines. Batches are processed in groups of 4 so the q projection streams
512-wide moving operands (4 batches x 128 queries per matmul).

The batch loop is software-pipelined: batch b's tail (normalize, transpose,
output projection) is emitted after batch b+1's projections+attention so the
in-order PE queue never head-of-line blocks on the Vector-engine tail chain.

Math note: the reference computes
    w = softmax(logits masked with -inf); w[nan] = 0
    w = w * diff; w = w / (sum(w) + 1e-8)
which equals
    num = exp(logits) * valid * diff
    w   = num / (sum(num) + 1e-8 * sum(exp(logits) * valid))
Folding the 1e-8 into the mask: M = valid * (diff + 1e-8) gives
    w ~= exp(logits) * M / sum(exp(logits) * M)
with an O(1e-8) absolute perturbation on w (negligible vs bf16 rounding).
Fully-masked rows: numerator is exactly 0 and the denominator gets +1e-25,
so those rows come out exactly 0, matching the reference's NaN->0 path.
"""

import numpy as np
import ml_dtypes

BS, NE, NQ = 128, 512, 128
DIN, EMB, ODIM = 512, 512, 512
H, HD = 8, 64
NCORES = 8
BPC = BS // NCORES          # batches per core
GRP = 4                     # batches per q-projection group
EC = DIN // 128             # contraction chunks (4)
BF16 = ml_dtypes.bfloat16


def _build_nc():
    import concourse.bacc as bacc
    import concourse.mybir as mybir
    import concourse.tile as tile
    from concourse.masks import make_identity

    f32 = mybir.dt.float32
    bf16 = mybir.dt.bfloat16

    nc = bacc.Bacc("TRN2", target_bir_lowering=False, debug=False,
                   num_devices=NCORES)

    ents_d = nc.dram_tensor("entsT", [BPC, DIN, NE], bf16, kind="ExternalInput")
    mask_d = nc.dram_tensor("maskT", [BPC, NE, NQ], bf16, kind="ExternalInput")
    win_d = nc.dram_tensor("w_inT", [DIN, 3 * EMB], bf16, kind="ExternalInput")
    wout_d = nc.dram_tensor("w_outT", [EMB, ODIM], bf16, kind="ExternalInput")
    pm_d = nc.dram_tensor("pmT", [NQ, BPC], f32, kind="ExternalInput")
    out_d = nc.dram_tensor("out", [BPC, NQ, ODIM], f32, kind="ExternalOutput")

    with tile.TileContext(nc) as tc:
        with (
            tc.tile_pool(name="const", bufs=1) as cpool,
            tc.tile_pool(name="gwork", bufs=2) as gwork,
            tc.tile_pool(name="work", bufs=3) as work,
            tc.tile_pool(name="nums", bufs=6) as nums,
            tc.tile_pool(name="ps", bufs=6, space="PSUM") as ps,
            tc.tile_pool(name="ps_att", bufs=2, space="PSUM") as ps_att,
        ):
            # ---- constants (issue order matters: batch 0's k-projection
            # needs only w_in chunk 0 + the first entity slab, so those two
            # DMAs go first and the rest follows the first entity DMA) ----
            ident = cpool.tile([128, 128], bf16)
            make_identity(nc, ident)
            w_in_sb = cpool.tile([128, EC, 3 * EMB], bf16)
            win_r = win_d.ap().rearrange("(c p) f -> p c f", p=128)
            nc.sync.dma_start(out=w_in_sb[:, 0, :], in_=win_r[:, 0, :])
            w_out_sb = cpool.tile([128, EC, ODIM], bf16)
            pm_sb = cpool.tile([128, BPC], f32)
            nc.gpsimd.dma_start(out=pm_sb, in_=pm_d.ap())

            def late_consts():
                for ce in range(1, EC):
                    nc.sync.dma_start(out=w_in_sb[:, ce, :], in_=win_r[:, ce, :])
                nc.sync.dma_start(
                    out=w_out_sb,
                    in_=wout_d.ap().rearrange("(c p) f -> p c f", p=128))

            # warm-up matmuls: keep the PE HAM busy while the first weight
            # and entity DMAs are in flight so real matmuls start at 2.4 GHz
            psum_w = ps.tile([128, 128], f32, tag="big", name="psum_w")
            for _ in range(30):
                nc.tensor.matmul(psum_w, lhsT=ident, rhs=ident,
                                 start=True, stop=True)

            def load_group(g):
                ents_sb = gwork.tile([128, EC, GRP, NE], bf16, name="ents_sb")
                for i in range(GRP):
                    nc.sync.dma_start(
                        out=ents_sb[:, :, i, :],
                        in_=ents_d.ap()[g * GRP + i]
                            .rearrange("(c p) n -> p c n", p=128))
                    if g == 0 and i == 0:
                        late_consts()
                return ents_sb

            def qproj(ents_sb):
                # fused qT projection for the whole group:
                # qT4[f, i, q] , i = batch within group
                qT_sb = gwork.tile([128, 4, GRP, NQ], bf16, name="qT_sb")
                for cf in range(4):
                    psum_q = ps.tile([128, GRP, NQ], f32, tag="big",
                                     name="psum_q")
                    for ce in range(EC):
                        nc.tensor.matmul(
                            psum_q,
                            lhsT=w_in_sb[:, ce, 128 * cf:128 * (cf + 1)],
                            rhs=ents_sb[:, ce, :, 0:NQ],
                            start=(ce == 0), stop=(ce == EC - 1))
                    if cf % 2 == 0:
                        nc.scalar.copy(qT_sb[:, cf, :, :], psum_q)
                    else:
                        nc.vector.tensor_copy(qT_sb[:, cf, :, :], psum_q)
                return qT_sb

            def head_kv(b, ents_sb, i):
                """k/v projections for batch b (slot i in group)."""
                mask_sb = work.tile([128, EC, NQ], bf16, name="mask_sb")
                nc.gpsimd.dma_start(
                    out=mask_sb,
                    in_=mask_d.ap()[b].rearrange("(c p) q -> p c q", p=128))

                # kT projection: kT[f, n]
                kT_sb = work.tile([128, 4, NE], bf16, name="kT_sb")
                for cf in range(4):
                    psum_k = ps.tile([128, NE], f32, tag="big", name="psum_k")
                    for ce in range(EC):
                        nc.tensor.matmul(
                            psum_k,
                            lhsT=w_in_sb[:, ce, EMB + 128 * cf:EMB + 128 * (cf + 1)],
                            rhs=ents_sb[:, ce, i, :],
                            start=(ce == 0), stop=(ce == EC - 1))
                    if cf % 2 == 0:
                        nc.scalar.copy(kT_sb[:, cf, :], psum_k)
                    else:
                        nc.vector.tensor_copy(kT_sb[:, cf, :], psum_k)

                # v projection (natural layout) + ones column
                v_sb = work.tile([128, 4, H, HD + 1], bf16, name="v_sb")
                nc.gpsimd.memset(v_sb[:, :, :, HD], 1.0)
                for cn in range(4):
                    psum_v = ps.tile([128, EMB], f32, tag="big", name="psum_v")
                    for ce in range(EC):
                        nc.tensor.matmul(
                            psum_v,
                            lhsT=ents_sb[:, ce, i, 128 * cn:128 * (cn + 1)],
                            rhs=w_in_sb[:, ce, 2 * EMB:3 * EMB],
                            start=(ce == 0), stop=(ce == EC - 1))
                    src = psum_v.rearrange("p (h d) -> p h d", h=H)
                    if cn % 2 == 0:
                        nc.scalar.copy(v_sb[:, cn, :, 0:HD], src)
                    else:
                        nc.vector.tensor_copy(v_sb[:, cn, :, 0:HD], src)
                return mask_sb, kT_sb, v_sb

            def head_attn(b, qT_sb, i, mask_sb, kT_sb, v_sb):
                # attention, head pairs interleaved so the k=64 logits matmuls
                # run concurrently in the PE array (row groups 0-63 / 64-127)
                att_tiles = [
                    ps_att.tile([128, 4, HD + 1], f32, tag="att", name="patt0"),
                    ps_att.tile([128, 4, HD + 1], f32, tag="att", name="patt1"),
                ]
                for hc in range(4):          # head pair (2*hc, 2*hc+1)
                    psl = [
                        ps.tile([128, 4, NQ], f32, tag="big", name="psl0"),
                        ps.tile([128, 4, NQ], f32, tag="big", name="psl1"),
                    ]
                    for cn in range(4):
                        for r in range(2):   # row group r*64
                            nc.tensor.matmul(
                                psl[r][:, cn, :],
                                lhsT=kT_sb[64 * r:64 * (r + 1), hc,
                                           128 * cn:128 * (cn + 1)],
                                rhs=qT_sb[64 * r:64 * (r + 1), hc, i, :],
                                start=True, stop=True)
                    for r in range(2):
                        h = 2 * hc + r
                        exp_sb = nums.tile([128, 4, NQ], bf16, tag="exp",
                                           name="exp_sb")
                        nc.scalar.activation(
                            exp_sb, psl[r],
                            mybir.ActivationFunctionType.Exp, scale=1.0 / 8.0)
                        num_sb = nums.tile([128, 4, NQ], bf16, tag="num",
                                           name="num_sb")
                        nc.vector.tensor_mul(num_sb, exp_sb, mask_sb)
                        patt, j = att_tiles[h // 4], h % 4
                        for cn in range(4):
                            nc.tensor.matmul(
                                patt[:, j, :],
                                lhsT=num_sb[:, cn, :],
                                rhs=v_sb[:, cn, h, :],
                                start=(cn == 0), stop=(cn == 3))
                return att_tiles

            def tail_v(att_tiles):
                """denominators + normalize (Vector only, no PE work).
                Emitted unskewed, right after the batch's attention, so the
                attention PSUM banks release a full batch earlier."""
                dall_sb = work.tile([128, H], f32, name="dall_sb")
                nc.vector.tensor_copy(dall_sb[:, 0:4], att_tiles[0][:, :, HD])
                nc.vector.tensor_copy(dall_sb[:, 4:8], att_tiles[1][:, :, HD])
                deps_sb = work.tile([128, H], f32, name="deps_sb")
                nc.vector.tensor_scalar_add(deps_sb, dall_sb, 1e-25)
                recip_sb = work.tile([128, H], f32, name="recip_sb")
                nc.vector.reciprocal(recip_sb, deps_sb)

                attn_sb = work.tile([128, EMB], bf16, name="attn_sb")
                for h in range(H):
                    nc.vector.tensor_scalar_mul(
                        attn_sb[:, HD * h:HD * (h + 1)],
                        att_tiles[h // 4][:, h % 4, 0:HD],
                        recip_sb[:, h:h + 1])
                return attn_sb

            def tail_pe(b, attn_sb):
                """transpose + output projection (skewed one batch)."""
                # transpose attn -> attnT[E, q] on the DMA xbar (keeps the
                # PE queue, the critical resource, free of transpose pairs);
                # the final batch splits across both HWDGE queues to halve
                # its fully-exposed serial chain
                attnT_sb = work.tile([128, 4, 128], bf16, name="attnT_sb")
                for ct in range(4):
                    eng = nc.scalar if (b == BPC - 1 and ct % 2) else nc.sync
                    eng.dma_start_transpose(
                        attnT_sb[:, ct, :], attn_sb[:, 128 * ct:128 * (ct + 1)])

                psum_o = ps.tile([128, ODIM], f32, tag="big", name="psum_o")
                for ct in range(4):
                    nc.tensor.matmul(
                        psum_o,
                        lhsT=attnT_sb[:, ct, :],
                        rhs=w_out_sb[:, ct, :],
                        start=(ct == 0), stop=(ct == 3))
                out_sb = work.tile([128, ODIM], f32, name="out_sb")
                nc.vector.tensor_scalar_mul(out_sb, psum_o, pm_sb[:, b:b + 1])
                nc.gpsimd.dma_start(out=out_d.ap()[b], in_=out_sb)

            pending = None
            ngrp = BPC // GRP
            ents_cur = load_group(0)
            for g in range(ngrp):
                ents_next = None
                qT_sb = None
                for i in range(GRP):
                    b = g * GRP + i
                    kv = head_kv(b, ents_cur, i)
                    if qT_sb is None:
                        qT_sb = qproj(ents_cur)
                    att = head_attn(b, qT_sb, i, *kv)
                    attn_sb = tail_v(att)
                    # prefetch next group's entities before this group's
                    # remaining tails occupy the sync DMA queue
                    if i == 2 and g + 1 < ngrp:
                        ents_next = load_group(g + 1)
                    if pending is not None:
                        tail_pe(b - 1, pending)
                    pending = attn_sb
                ents_cur = ents_next
            tail_pe(BPC - 1, pending)

    nc.compile()
    return nc


def _prep_inputs(entities, pre_mask, diff_mask, post_mask, W_in, W_out):
    entities = np.asarray(entities, dtype=np.float32)
    pre_mask = np.asarray(pre_mask, dtype=bool)
    diff_mask = np.asarray(diff_mask, dtype=np.float32)
    post_mask = np.asarray(post_mask, dtype=bool)
    W_in = np.asarray(W_in, dtype=np.float32)
    W_out = np.asarray(W_out, dtype=np.float32)

    entsT = np.ascontiguousarray(entities.transpose(0, 2, 1)).astype(BF16)
    m = (~pre_mask).astype(np.float32) * (diff_mask + 1e-8)
    maskT = np.ascontiguousarray(m.transpose(0, 2, 1)).astype(BF16)
    w_inT = np.ascontiguousarray(W_in.T).astype(BF16)
    w_outT = np.ascontiguousarray(W_out.T).astype(BF16)
    pmT = np.ascontiguousarray((~post_mask).T.astype(np.float32))

    in_maps = []
    for c in range(NCORES):
        sl = slice(c * BPC, (c + 1) * BPC)
        in_maps.append({
            "entsT": np.ascontiguousarray(entsT[sl]),
            "maskT": np.ascontiguousarray(maskT[sl]),
            "w_inT": w_inT,
            "w_outT": w_outT,
            "pmT": np.ascontiguousarray(pmT[:, sl]),
        })
    return in_maps


def _run(in_maps, trace=False):
    from concourse.bass_utils import run_bass_kernel_spmd
    nc = _build_nc()
    last_exc = None
    for attempt in range(3):
        try:
            return run_bass_kernel_spmd(
                nc, in_maps, core_ids=list(range(NCORES)), trace=trace)
        except Exception as e:  # transient NRT_EXEC_UNIT faults on fresh NEFFs
            last_exc = e
            import time
            time.sleep(2.0 * (attempt + 1))
    raise last_exc


def kernel_traced(entities, pre_mask, diff_mask, post_mask, W_in, W_out, b_out,
                  trace=False):
    """Returns (output, BassKernelResults)."""
    b_out = np.asarray(b_out, dtype=np.float32)
    post_mask_np = np.asarray(post_mask, dtype=bool)
    in_maps = _prep_inputs(entities, pre_mask, diff_mask, post_mask, W_in, W_out)
    res = _run(in_maps, trace=trace)
    out = np.concatenate([r["out"] for r in res.results], axis=0)
    # faithfulness: reference adds b_out before the post-mask zeroing
    out = out + np.where(post_mask_np[:, :, None], 0.0, b_out[None, None, :])
    return out.astype(np.float32), res


def kernel(entities, pre_mask, diff_mask, post_mask, W_in, W_out, b_out):
    out, _ = kernel_traced(entities, pre_mask, diff_mask, post_mask,
                           W_in, W_out, b_out)
    return out

